# revision 3
# baseline (speedup 1.0000x reference)
"""Trainium2 Bass kernel for ContrastiveMultiTaskLoss.

Computes, on 8 NeuronCores (SPMD, no collectives):
  loss = 1.0*mse(price) + 0.5*mse(change) + 0.3*bce(crit)
       + 0.1 * NT-Xent(z1, z2, temp=0.1)

Strategy: every core receives the full z1/z2 ([8192,256] f32) plus a
row-block of queries zq ([2048,256]) and small per-core slices for the
positive-pair / supervised partial sums.  Each core:
  - normalizes all 16384 embedding rows (fp32 math), casts to bf16 and
    transposes them on the PE into a resident SBUF layout znT[K=2][128, 16384]
  - normalizes+transposes its 2048 query rows the same way (bit-identical
    values since the pipeline is identical)
  - computes its [2048, 16384] slice of the similarity matrix with bf16
    matmuls (K=256 in 2 accumulating steps, N=512 per PSUM bank), and
    in the same pipeline exponentiates each [128,2048] PSUM tile on the
    scalar engine (func=Exp, scale=1/temp) with accum_out giving the
    row sums of exp directly
  - subtracts exp(self-sim) (from ||zn_bf16||^2, matching the matmul diag
    to fp32 rounding), takes Ln, and accumulates log-sum-exp partials
  - computes positive-pair dot partials and the supervised loss partials
The host sums the 8 [128,8] partial tensors and applies the loss weights.
"""

import numpy as np

import concourse.bass as bass
import concourse.mybir as mybir
import concourse.tile as tile
from concourse import bacc
from concourse.bass_utils import run_bass_kernel_spmd
from concourse.masks import make_identity

F32 = mybir.dt.float32
BF16 = mybir.dt.bfloat16
AF = mybir.ActivationFunctionType

N_CORES = 8
D = 256
KH = 2           # K halves (D = 2*128)
GCOLS = 2048     # columns per sim-group (4 PSUM banks of 512 f32)
ITEMP = 10.0     # 1/temperature
W_PRICE, W_CHANGE, W_CRIT = 1.0, 0.5, 0.3
SSL_WEIGHT = 0.1


class Cfg:
    def __init__(self, n):
        self.n = n                       # rows in z1 (= rows in z2)
        self.two_n = 2 * n
        self.rows_q = 2 * n // N_CORES   # query rows per core
        self.n_rowtiles = self.rows_q // 128
        self.n_groups = self.two_n // GCOLS
        self.n_big = n // 1024           # 1024-row load tiles per z tensor
        self.pos_rows = n // N_CORES     # pos-pair rows per core
        self.pa = self.pos_rows // 128   # chunks of 128
        self.sup_rows = n // N_CORES
        self.sa = self.sup_rows // 128


FULL = Cfg(8192)


def build_program(cfg, repeat=1):
    nc = bacc.Bacc("TRN2", target_bir_lowering=False, debug=False,
                   num_devices=N_CORES)
    z1_ext = nc.dram_tensor("z1", [cfg.n, D], F32, kind="ExternalInput")
    z2_ext = nc.dram_tensor("z2", [cfg.n, D], F32, kind="ExternalInput")
    zq_ext = nc.dram_tensor("zq", [cfg.rows_q, D], F32, kind="ExternalInput")
    zp1_ext = nc.dram_tensor("zp1", [cfg.pos_rows, D], F32, kind="ExternalInput")
    zp2_ext = nc.dram_tensor("zp2", [cfg.pos_rows, D], F32, kind="ExternalInput")
    sup_ext = nc.dram_tensor("sup", [6, cfg.sup_rows], F32, kind="ExternalInput")
    part_ext = nc.dram_tensor("partials", [128, 8], F32, kind="ExternalOutput")

    with tile.TileContext(nc) as tc:
        for _ in range(repeat):
            _emit(nc, tc, cfg, z1_ext, z2_ext, zq_ext, zp1_ext, zp2_ext,
                  sup_ext, part_ext)
    nc.compile()
    return nc


def _emit(nc, tc, cfg, z1_ext, z2_ext, zq_ext, zp1_ext, zp2_ext,
          sup_ext, part_ext):
    from contextlib import ExitStack
    ctx = ExitStack()
    with ctx:
        singles = ctx.enter_context(tc.tile_pool(name="singles", bufs=1))
        loads = ctx.enter_context(tc.tile_pool(name="loads", bufs=3))
        stage = ctx.enter_context(tc.tile_pool(name="stage", bufs=2))
        small = ctx.enter_context(tc.tile_pool(name="small", bufs=4))
        dumps = ctx.enter_context(tc.tile_pool(name="dumps", bufs=2))

        ident = singles.tile([128, 128], BF16, tag="ident")
        make_identity(nc, ident[:])

        partials = singles.tile([128, 8], F32, tag="partials")
        nc.vector.memset(partials[:, 5:8], 0.0)

        # resident transposed bf16 embeddings: znt[h][g] = [128, GCOLS]
        znt = [[singles.tile([128, GCOLS], BF16, tag=f"znt_{h}_{g}",
                             name=f"znt_{h}_{g}")
                for g in range(cfg.n_groups)] for h in range(KH)]
        # resident transposed bf16 queries: znqt[h] = [128, rows_q]
        znqt = [singles.tile([128, cfg.rows_q], BF16, tag=f"znqt_{h}",
                             name=f"znqt_{h}")
                for h in range(KH)]
        # exp(self-sim) per query row: [128, n_rowtiles]
        d2q = singles.tile([128, cfg.n_rowtiles], F32, tag="d2q")
        expdq = singles.tile([128, cfg.n_rowtiles], F32, tag="expdq")
        loghold = singles.tile([128, cfg.n_rowtiles], F32, tag="loghold")

        # ---------------- prologue: normalize + transpose -------------
        def norm_cast(big_rows_ap, nbig_chunks, out_bf):
            """Load+normalize a [128, A, 256] f32 tile; write bf16 zn.

            big_rows_ap: DRAM AP [128, A, 256]; out_bf: SBUF bf16 tile.
            Returns nothing; out_bf filled.
            """
            A = nbig_chunks
            zbig = loads.tile([128, A, D], F32, tag="zbig")
            nc.sync.dma_start(out=zbig[:], in_=big_rows_ap)
            n2 = small.tile([128, A], F32, tag="n2")
            sqd = stage.tile([128, D], F32, tag="sqd")
            for a in range(A):
                nc.scalar.activation(out=sqd[:], in_=zbig[:, a, :],
                                     func=AF.Square, accum_out=n2[:, a:a + 1])
            lnn = small.tile([128, A], F32, tag="lnn")
            nc.scalar.activation(out=lnn[:], in_=n2[:], func=AF.Ln)
            rn = small.tile([128, A], F32, tag="rn")
            nc.scalar.activation(out=rn[:], in_=lnn[:], func=AF.Exp, scale=-0.5)
            for a in range(A):
                nc.vector.tensor_scalar_mul(out_bf[:, a, :], zbig[:, a, :],
                                            rn[:, a:a + 1])

        def transpose_chunks(zn_bf, A, dest_fn, tpsum, start_chunk):
            """PE-transpose [128,256] chunks into destination slices.

            dest_fn(h, chunk_idx) -> (tile, col_offset)"""
            for a in range(A):
                for h in range(KH):
                    pt = tpsum.tile([128, 128], BF16, tag="pt")
                    nc.tensor.transpose(pt[:], zn_bf[:, a, h * 128:(h + 1) * 128],
                                        ident[:])
                    dst, off = dest_fn(h, start_chunk + a)
                    if (start_chunk + a) % 2 == 0:
                        nc.scalar.copy(dst[:, off:off + 128], pt[:])
                    else:
                        nc.vector.tensor_copy(dst[:, off:off + 128], pt[:])

        with tc.tile_pool(name="tpsum", bufs=4, space="PSUM") as tpsum:
            # keys: z1 then z2
            for zi, z_ext in enumerate((z1_ext, z2_ext)):
                zr = z_ext.ap().rearrange("(c p) d -> p c d", p=128)
                nchunks = cfg.n // 128
                for start in range(0, nchunks, 8):
                    A = min(8, nchunks - start)
                    znb = stage.tile([128, A, D], BF16, tag="znb")
                    norm_cast(zr[:, start:start + A, :], A, znb)
                    base_chunk = zi * nchunks + start

                    def kdest(h, chunk):
                        g, c = divmod(chunk, GCOLS // 128)
                        return znt[h][g], c * 128

                    transpose_chunks(znb, A, kdest, tpsum, base_chunk)

            # queries
            zqr = zq_ext.ap().rearrange("(c p) d -> p c d", p=128)
            nqchunks = cfg.rows_q // 128
            for start in range(0, nqchunks, 8):
                A = min(8, nqchunks - start)
                znb = stage.tile([128, A, D], BF16, tag="znb")
                norm_cast(zqr[:, start:start + A, :], A, znb)
                # self-sim norm^2 of the bf16 values
                sqd = stage.tile([128, D], F32, tag="sqd")
                for a in range(A):
                    m = start + a
                    nc.scalar.activation(out=sqd[:], in_=znb[:, a, :],
                                         func=AF.Square,
                                         accum_out=d2q[:, m:m + 1])

                def qdest(h, chunk):
                    return znqt[h], chunk * 128

                transpose_chunks(znb, A, qdest, tpsum, start)

            nc.scalar.activation(out=expdq[:], in_=d2q[:], func=AF.Exp,
                                 scale=ITEMP)

            # ---- positive pairs + supervised (tiny) ----
            _emit_pos_sup(nc, tc, cfg, zp1_ext, zp2_ext, sup_ext,
                          partials, loads, stage, small)

        # ---------------- main loop: sim + exp + rowsums --------------
        with tc.tile_pool(name="mpsum", bufs=2, space="PSUM") as mpsum:
            for m in range(cfg.n_rowtiles):
                acc = small.tile([128, cfg.n_groups], F32, tag="acc")
                for g in range(cfg.n_groups):
                    pt = mpsum.tile([128, GCOLS], F32, tag="mp")
                    for h in range(KH):
                        for j in range(GCOLS // 512):
                            nc.tensor.matmul(
                                pt[:, j * 512:(j + 1) * 512],
                                lhsT=znqt[h][:, m * 128:(m + 1) * 128],
                                rhs=znt[h][g][:, j * 512:(j + 1) * 512],
                                start=(h == 0), stop=(h == KH - 1))
                    edump = dumps.tile([128, GCOLS], BF16, tag="edump")
                    nc.scalar.activation(out=edump[:], in_=pt[:], func=AF.Exp,
                                         scale=ITEMP, accum_out=acc[:, g:g + 1])
                rs = small.tile([128, 1], F32, tag="rs")
                accdump = small.tile([128, cfg.n_groups], F32, tag="accdump")
                nc.scalar.activation(out=accdump[:], in_=acc[:],
                                     func=AF.Identity, accum_out=rs[:])
                rsc = small.tile([128, 1], F32, tag="rsc")
                nc.vector.tensor_sub(rsc[:], rs[:], expdq[:, m:m + 1])
                nc.scalar.activation(out=loghold[:, m:m + 1], in_=rsc[:],
                                     func=AF.Ln)

        lhdump = small.tile([128, cfg.n_rowtiles], F32, tag="lhdump")
        nc.scalar.activation(out=lhdump[:], in_=loghold[:], func=AF.Identity,
                             accum_out=partials[:, 0:1])
        nc.sync.dma_start(out=part_ext[:], in_=partials[:])


def _emit_pos_sup(nc, tc, cfg, zp1_ext, zp2_ext, sup_ext, partials,
                  loads, stage, small):
    A = cfg.pa
    # --- positive pair partial: sum over rows of zn1 . zn2 (unscaled) ---
    zp1r = zp1_ext.ap().rearrange("(a p) d -> p a d", p=128)
    zp2r = zp2_ext.ap().rearrange("(a p) d -> p a d", p=128)
    p1 = loads.tile([128, A, D], F32, tag="p1")
    p2 = loads.tile([128, A, D], F32, tag="p2")
    nc.sync.dma_start(out=p1[:], in_=zp1r)
    nc.sync.dma_start(out=p2[:], in_=zp2r)
    n2a = small.tile([128, A], F32, tag="n2a")
    n2b = small.tile([128, A], F32, tag="n2b")
    dots = small.tile([128, A], F32, tag="dots")
    sqd = stage.tile([128, D], F32, tag="psq")
    prod = stage.tile([128, A, D], F32, tag="prod")
    nc.vector.tensor_mul(prod[:], p1[:], p2[:])
    for a in range(A):
        nc.scalar.activation(out=sqd[:], in_=p1[:, a, :], func=AF.Square,
                             accum_out=n2a[:, a:a + 1])
        nc.scalar.activation(out=sqd[:], in_=p2[:, a, :], func=AF.Square,
                             accum_out=n2b[:, a:a + 1])
        nc.scalar.activation(out=sqd[:], in_=prod[:, a, :], func=AF.Identity,
                             accum_out=dots[:, a:a + 1])
    lna = small.tile([128, A], F32, tag="lna")
    lnb = small.tile([128, A], F32, tag="lnb")
    nc.scalar.activation(out=lna[:], in_=n2a[:], func=AF.Ln)
    nc.scalar.activation(out=lnb[:], in_=n2b[:], func=AF.Ln)
    lnsum = small.tile([128, A], F32, tag="lnsum")
    nc.vector.tensor_add(lnsum[:], lna[:], lnb[:])
    rp = small.tile([128, A], F32, tag="rp")
    nc.scalar.activation(out=rp[:], in_=lnsum[:], func=AF.Exp, scale=-0.5)
    pos = small.tile([128, A], F32, tag="pos")
    nc.vector.tensor_mul(pos[:], dots[:], rp[:])
    pdump = small.tile([128, A], F32, tag="pdump")
    nc.scalar.activation(out=pdump[:], in_=pos[:], func=AF.Identity,
                         accum_out=partials[:, 1:2])

    # --- supervised partials ---
    S = cfg.sa
    supr = sup_ext.ap().rearrange("s (p a) -> p s a", p=128)
    sup = loads.tile([128, 6, S], F32, tag="sup")
    nc.sync.dma_start(out=sup[:], in_=supr)
    d8 = small.tile([128, S], F32, tag="d8")
    sdump = small.tile([128, S], F32, tag="sdump")
    nc.vector.tensor_sub(d8[:], sup[:, 0, :], sup[:, 1, :])
    nc.scalar.activation(out=sdump[:], in_=d8[:], func=AF.Square,
                         accum_out=partials[:, 2:3])
    d8b = small.tile([128, S], F32, tag="d8b")
    nc.vector.tensor_sub(d8b[:], sup[:, 2, :], sup[:, 3, :])
    nc.scalar.activation(out=sdump[:], in_=d8b[:], func=AF.Square,
                         accum_out=partials[:, 3:4])
    # bce: relu(x) - x*t + ln(1 + exp(-|x|))
    x_ap = sup[:, 4, :]
    t_ap = sup[:, 5, :]
    r8 = small.tile([128, S], F32, tag="r8")
    nc.scalar.activation(out=r8[:], in_=x_ap, func=AF.Relu)
    a8 = small.tile([128, S], F32, tag="a8")
    nc.scalar.activation(out=a8[:], in_=x_ap, func=AF.Abs)
    e8 = small.tile([128, S], F32, tag="e8")
    nc.scalar.activation(out=e8[:], in_=a8[:], func=AF.Exp, scale=-1.0)
    l8 = small.tile([128, S], F32, tag="l8")
    nc.scalar.activation(out=l8[:], in_=e8[:], func=AF.Ln, bias=1.0)
    xt8 = small.tile([128, S], F32, tag="xt8")
    nc.vector.tensor_mul(xt8[:], x_ap, t_ap)
    s1 = small.tile([128, S], F32, tag="s1")
    nc.vector.tensor_add(s1[:], r8[:], l8[:])
    s2 = small.tile([128, S], F32, tag="s2")
    nc.vector.tensor_sub(s2[:], s1[:], xt8[:])
    nc.scalar.activation(out=sdump[:], in_=s2[:], func=AF.Identity,
                         accum_out=partials[:, 4:5])


def make_in_maps(cfg, price_pred, price_target, change_pred, change_target,
                 criticality_pred, criticality_target, z1, z2):
    z1 = np.ascontiguousarray(np.asarray(z1, dtype=np.float32))
    z2 = np.ascontiguousarray(np.asarray(z2, dtype=np.float32))
    sups = [np.asarray(a, dtype=np.float32).reshape(-1) for a in
            (price_pred, price_target, change_pred, change_target,
             criticality_pred, criticality_target)]
    in_maps = []
    rq = cfg.rows_q
    pr = cfg.pos_rows
    for c in range(N_CORES):
        qstart = c * rq
        if qstart < cfg.n:
            zq = z1[qstart:qstart + rq]
        else:
            zq = z2[qstart - cfg.n:qstart - cfg.n + rq]
        sl = slice(c * pr, (c + 1) * pr)
        sup = np.stack([s[c * cfg.sup_rows:(c + 1) * cfg.sup_rows]
                        for s in sups])
        in_maps.append({
            "z1": z1, "z2": z2,
            "zq": np.ascontiguousarray(zq),
            "zp1": np.ascontiguousarray(z1[sl]),
            "zp2": np.ascontiguousarray(z2[sl]),
            "sup": np.ascontiguousarray(sup),
        })
    return in_maps


def combine(cfg, results):
    cols = np.zeros(8, dtype=np.float64)
    for r in results:
        cols += r["partials"].astype(np.float64).sum(axis=0)
    slog, sdot, sprice, schange, scrit = cols[0], cols[1], cols[2], cols[3], cols[4]
    n = float(cfg.n)
    ssl = (slog - 2.0 * ITEMP * sdot) / (2.0 * n)
    supervised = (W_PRICE * sprice + W_CHANGE * schange + W_CRIT * scrit) / n
    return np.float32(supervised + SSL_WEIGHT * ssl)


_compiled = {}


def _get_program(repeat=1):
    key = repeat
    if key not in _compiled:
        _compiled[key] = build_program(FULL, repeat=repeat)
    return _compiled[key]


def kernel(**inputs):
    nc = _get_program()
    in_maps = make_in_maps(FULL, **inputs)
    res = run_bass_kernel_spmd(nc, in_maps, list(range(N_CORES)))
    return combine(FULL, res.results)
